# revision 5
# baseline (speedup 1.0000x reference)
"""LocallyConnected2dV2 Trainium2 kernel (bf16, dual-engine DMA issue).

Math: out[b, a, bp] = sum_{k,l} xpad[b, 5a+k, 5bp+l] * kw[a, bp, k, l] + bias[a, bp]
where xpad is x zero-padded by 2, kw is extracted from the sparse-structured
W[17424, 625] (100 nonzeros per column at statically known positions).

Strategy (8 cores, data-parallel over batch, 128 images/core):
  - Host: transpose each core's x shard to [col j', row r, batch b] so the
    contraction dim (image columns) lands on SBUF partitions; cast bf16.
  - Host: compact W into per-image-row banded blocks Wh[r, j', 50] (bf16).
  - Device: for each output-row group g (5 groups x 5 a's = 125 psum cols),
    accumulate over the ~30 contributing image rows:
      psum[b, nk] += xT_row[j', b].T @ Wh_row[j', 50]   (bf16 matmuls, fp32 psum)
    Bias enters as the last matmul of each group via a K=1 ones-vector matmul.
  - DMA: combined x+w chunks (last two half-size to shorten the post-stream
    tail), issue split between the two HWDGE engines (sync/scalar). Output
    streams out per group as bf16; the last group goes out in two halves.
"""

import numpy as np
import ml_dtypes

BF16 = ml_dtypes.bfloat16

B = 1024
R = 128           # image rows = cols
NCORES = 8
BS = B // NCORES  # 128 batch per core
NK = 625
WP = 132
NG = 5            # output-row groups (5 a's each)
GW = 125          # psum cols per group

# chunk row spans: 7x16 rows + 2x8 rows (small tail chunks)
CHUNK_OFF = [0, 16, 32, 48, 64, 80, 96, 112, 120]
CHUNK_LEN = [16, 16, 16, 16, 16, 16, 16, 8, 8]
NCH = len(CHUNK_OFF)


def _chunk_of_row(r):
    for ic in range(NCH):
        if CHUNK_OFF[ic] <= r < CHUNK_OFF[ic] + CHUNK_LEN[ic]:
            return ic
    raise ValueError(r)


def _a0_of_row(r):
    return min(max((r - 3) // 5, 0), 23)


def _group_rows(g):
    return range(max(0, 25 * g - 2), min(R - 1, 25 * g + 27) + 1)


def _row_parts(r, g):
    """Matmul pieces row r contributes to group g.

    Returns list of (psum_col, n_cols, w_col): psum slice [psum_col, +n),
    weight slice [w_col, +n) within the row's 50-wide weight block.
    """
    a0 = _a0_of_row(r)
    lo = 5 * g
    if a0 >= lo and a0 + 1 < lo + 5:
        return [((a0 - lo) * 25, 50, 0)]
    parts = []
    for ai, a in ((0, a0), (1, a0 + 1)):
        if lo <= a < lo + 5:
            parts.append(((a - lo) * 25, 25, ai * 25))
    return parts


def prep_weights(W, bias):
    """W [17424, 625], bias [25,25] -> wt [128, 128*50] ([j'][r, c] layout),
    bs [1, 625] (fp32; cast to bf16 at pack time)."""
    W = np.asarray(W, np.float32)
    i = np.arange(NK)
    si = (i // 25) * 5
    sj = (i % 25) * 5
    rows = ((si[:, None, None] + np.arange(10)[None, :, None]) * WP
            + sj[:, None, None] + np.arange(10)[None, None, :])
    kw = W[rows.reshape(NK, 100), i[:, None]].reshape(25, 25, 10, 10)

    r = np.arange(R)[:, None, None]
    jp = np.arange(R)[None, :, None]
    c = np.arange(50)[None, None, :]
    ai = c // 25
    bp = c % 25
    a = np.clip((r - 3) // 5, 0, 23) + ai
    k = r + 2 - 5 * a
    l = jp + 2 - 5 * bp
    valid = (k >= 0) & (k < 10) & (l >= 0) & (l < 10)
    Wh = np.where(valid, kw[a, bp, np.clip(k, 0, 9), np.clip(l, 0, 9)], 0.0)
    Wh = Wh.astype(np.float32)                       # [r, j', 50]
    wt = np.ascontiguousarray(Wh.transpose(1, 0, 2)).reshape(R, R * 50)
    bs = np.ascontiguousarray(np.asarray(bias, np.float32).reshape(1, NK))
    return wt, bs


def _build_nc():
    import concourse.bass as bass
    import concourse.mybir as mybir
    import concourse.tile as tile
    from concourse import bacc

    bf16 = mybir.dt.bfloat16
    nc = bacc.Bacc("TRN2", target_bir_lowering=False, debug=False)
    xw_cols = sum(n * (BS + 50) for n in CHUNK_LEN)
    xw = nc.dram_tensor("xw", [R, xw_cols], bf16, kind="ExternalInput").ap()
    aux = nc.dram_tensor("aux", [1, NK + BS], bf16, kind="ExternalInput").ap()
    out = nc.dram_tensor("out", [NG * BS, GW], bf16, kind="ExternalOutput").ap()

    with tile.TileContext(nc) as tc:
        with (
            tc.tile_pool(name="xw", bufs=1) as xw_pool,
            tc.tile_pool(name="small", bufs=1) as small,
            tc.tile_pool(name="ps", bufs=5, space=bass.MemorySpace.PSUM) as ps_pool,
            tc.tile_pool(name="ob", bufs=1) as ob_pool,
        ):
            aux_t = small.tile([1, NK + BS], bf16, tag="aux")
            bias_t = aux_t[:, 0:NK]
            ones_t = aux_t[:, NK:NK + BS]

            # issue order: sync gets even chunks, scalar gets odd; aux after
            # scalar's first chunk so chunk 1 streams immediately
            ch = []
            col0 = 0
            for ic in range(NCH):
                cw = CHUNK_LEN[ic] * (BS + 50)
                t = xw_pool.tile([R, cw], bf16, tag=f"c{ic}")
                eng = nc.sync if ic % 2 == 0 else nc.scalar
                eng.dma_start(t[:], xw[:, col0:col0 + cw])
                if ic == 1:
                    nc.scalar.dma_start(aux_t[:], aux[:])
                ch.append(t)
                col0 += cw

            out_sb = ob_pool.tile([BS, NK], bf16, tag="osb")

            def do_group(g, splits):
                ps = ps_pool.tile([BS, GW], mybir.dt.float32, tag="ps")
                mms = []
                for r in _group_rows(g):
                    ic = _chunk_of_row(r)
                    rr = r - CHUNK_OFF[ic]
                    ct = ch[ic]
                    lhsT = ct[:, rr * BS:(rr + 1) * BS]
                    wb = CHUNK_LEN[ic] * BS + rr * 50
                    for (pc, n, wc) in _row_parts(r, g):
                        mms.append((ps[:, pc:pc + n], lhsT,
                                    ct[:, wb + wc:wb + wc + n]))
                # bias enters last so aux stays off the group-start path
                mms.append((ps[:, 0:GW], ones_t, bias_t[:, g * GW:(g + 1) * GW]))
                last = len(mms) - 1
                for idx, (o, lh, rh) in enumerate(mms):
                    nc.tensor.matmul(o, lh, rh,
                                     start=(idx == 0), stop=(idx == last))
                for (c0, c1) in splits:
                    nc.vector.tensor_copy(
                        out_sb[:, g * GW + c0:g * GW + c1], ps[:, c0:c1])
                    nc.scalar.dma_start(
                        out[g * BS:(g + 1) * BS, c0:c1],
                        out_sb[:, g * GW + c0:g * GW + c1])

            for g in range(NG - 1):
                do_group(g, [(0, GW)])
            do_group(NG - 1, [(0, 75), (75, GW)])
    nc.compile()
    return nc


_NC_CACHE = []


def _get_nc():
    if not _NC_CACHE:
        _NC_CACHE.append(_build_nc())
    return _NC_CACHE[0]


def make_in_maps(x, W, bias):
    x = np.asarray(x, np.float32)
    wt, bsv = prep_weights(W, bias)
    wt16 = wt.astype(BF16)
    auxv = np.concatenate(
        [bsv.astype(BF16), np.ones((1, BS), BF16)], axis=1)
    in_maps = []
    for c in range(NCORES):
        xc = x[c * BS:(c + 1) * BS]                      # [b, r, j']
        xtv = np.ascontiguousarray(
            xc.transpose(2, 1, 0)).astype(BF16).reshape(R, R * BS)
        parts = []
        for ic in range(NCH):
            o, n = CHUNK_OFF[ic], CHUNK_LEN[ic]
            parts.append(xtv[:, o * BS:(o + n) * BS])
            parts.append(wt16[:, o * 50:(o + n) * 50])
        xwv = np.ascontiguousarray(np.concatenate(parts, axis=1))
        in_maps.append({"xw": xwv, "aux": auxv})
    return in_maps


def run(x, W, bias, trace=False, **kw):
    from concourse import bass_utils
    nc = _get_nc()
    res = bass_utils.run_bass_kernel_spmd(
        nc, make_in_maps(x, W, bias), list(range(NCORES)), trace=trace, **kw)
    outs = []
    for c in range(NCORES):
        o = np.asarray(res.results[c]["out"])            # [NG*BS, GW] bf16
        o = o.reshape(NG, BS, GW).transpose(1, 0, 2)     # [BS, NG, GW]
        outs.append(o.reshape(BS, 25, 25).astype(np.float32))
    return np.concatenate(outs, axis=0), res


def kernel(**inputs):
    out, _ = run(inputs["x"], inputs["W"], inputs["bias"])
    return out


# revision 9
# speedup vs baseline: 1.0867x; 1.0867x over previous
"""LocallyConnected2dV2 Trainium2 kernel (bf16, dual-engine DMA issue).

Math: out[b, a, bp] = sum_{k,l} xpad[b, 5a+k, 5bp+l] * kw[a, bp, k, l] + bias[a, bp]
where xpad is x zero-padded by 2, kw is extracted from the sparse-structured
W[17424, 625] (100 nonzeros per column at statically known positions).

Strategy (8 cores, data-parallel over batch, 128 images/core):
  - Host: transpose each core's x shard to [col j', row r, batch b] so the
    contraction dim (image columns) lands on SBUF partitions; cast bf16.
  - Host: compact W into per-image-row banded blocks Wh[r, j', 50] (bf16).
  - Device: for each output-row group g (5 groups x 5 a's = 125 psum cols),
    accumulate over the ~30 contributing image rows:
      psum[b, nk] += xT_row[j', b].T @ Wh_row[j', 50]   (bf16 matmuls, fp32 psum)
    Bias enters as the last matmul of each group via a K=1 ones-vector matmul.
  - DMA: 8 combined x+w chunks, issue alternating between the two HWDGE
    engines (sync/scalar) to halve serialized descriptor-gen time. Output
    streams out per group as bf16 to a group-major DRAM buffer.
"""

import numpy as np
import ml_dtypes

BF16 = ml_dtypes.bfloat16

B = 1024
R = 128           # image rows = cols
NCORES = 8
BS = B // NCORES  # 128 batch per core
NK = 625
WP = 132
NG = 5            # output-row groups (5 a's each)
GW = 125          # psum cols per group
CHUNK = 16        # image rows per DMA chunk
NCH = R // CHUNK


def _a0_of_row(r):
    return min(max((r - 3) // 5, 0), 23)


def _group_rows(g):
    return range(max(0, 25 * g - 2), min(R - 1, 25 * g + 27) + 1)


def _row_parts(r, g):
    """Matmul pieces row r contributes to group g.

    Returns list of (psum_col, n_cols, w_col): psum slice [psum_col, +n),
    weight slice [w_col, +n) within the row's 50-wide weight block.
    """
    a0 = _a0_of_row(r)
    lo = 5 * g
    if a0 >= lo and a0 + 1 < lo + 5:
        return [((a0 - lo) * 25, 50, 0)]
    parts = []
    for ai, a in ((0, a0), (1, a0 + 1)):
        if lo <= a < lo + 5:
            parts.append(((a - lo) * 25, 25, ai * 25))
    return parts


def prep_weights(W, bias):
    """W [17424, 625], bias [25,25] -> wt [128, 128*50] ([j'][r, c] layout),
    bs [1, 625] (fp32; cast to bf16 at pack time)."""
    W = np.asarray(W, np.float32)
    i = np.arange(NK)
    si = (i // 25) * 5
    sj = (i % 25) * 5
    rows = ((si[:, None, None] + np.arange(10)[None, :, None]) * WP
            + sj[:, None, None] + np.arange(10)[None, None, :])
    kw = W[rows.reshape(NK, 100), i[:, None]].reshape(25, 25, 10, 10)

    r = np.arange(R)[:, None, None]
    jp = np.arange(R)[None, :, None]
    c = np.arange(50)[None, None, :]
    ai = c // 25
    bp = c % 25
    a = np.clip((r - 3) // 5, 0, 23) + ai
    k = r + 2 - 5 * a
    l = jp + 2 - 5 * bp
    valid = (k >= 0) & (k < 10) & (l >= 0) & (l < 10)
    Wh = np.where(valid, kw[a, bp, np.clip(k, 0, 9), np.clip(l, 0, 9)], 0.0)
    Wh = Wh.astype(np.float32)                       # [r, j', 50]
    wt = np.ascontiguousarray(Wh.transpose(1, 0, 2)).reshape(R, R * 50)
    bs = np.ascontiguousarray(np.asarray(bias, np.float32).reshape(1, NK))
    return wt, bs


CW = CHUNK * BS + CHUNK * 50   # combined x+w free cols per chunk


def _build_nc():
    import concourse.bass as bass
    import concourse.mybir as mybir
    import concourse.tile as tile
    from concourse import bacc

    bf16 = mybir.dt.bfloat16
    nc = bacc.Bacc("TRN2", target_bir_lowering=False, debug=False)
    xw = nc.dram_tensor("xw", [R, NCH * CW], bf16, kind="ExternalInput").ap()
    aux = nc.dram_tensor("aux", [1, NK + BS], bf16, kind="ExternalInput").ap()
    out = nc.dram_tensor("out", [NG * BS, GW], bf16, kind="ExternalOutput").ap()

    with tile.TileContext(nc) as tc:
        with (
            tc.tile_pool(name="xw", bufs=1) as xw_pool,
            tc.tile_pool(name="small", bufs=1) as small,
            tc.tile_pool(name="ps", bufs=5, space=bass.MemorySpace.PSUM) as ps_pool,
            tc.tile_pool(name="ob", bufs=1) as ob_pool,
        ):
            aux_t = small.tile([1, NK + BS], bf16, tag="aux")
            nc.scalar.dma_start(aux_t[:], aux[:])
            bias_t = aux_t[:, 0:NK]
            ones_t = aux_t[:, NK:NK + BS]

            ch = []
            for ic in range(NCH):
                t = xw_pool.tile([R, CW], bf16, tag=f"c{ic}")
                eng = nc.sync if ic % 2 == 0 else nc.scalar
                eng.dma_start(t[:], xw[:, ic * CW:(ic + 1) * CW])
                ch.append(t)

            out_sb = ob_pool.tile([BS, NK], bf16, tag="osb")
            for g in range(NG):
                ps = ps_pool.tile([BS, GW], mybir.dt.float32)
                mms = []
                for r in _group_rows(g):
                    ct = ch[r // CHUNK]
                    lhsT = ct[:, (r % CHUNK) * BS:(r % CHUNK + 1) * BS]
                    wb = CHUNK * BS + (r % CHUNK) * 50
                    for (pc, n, wc) in _row_parts(r, g):
                        mms.append((ps[:, pc:pc + n], lhsT,
                                    ct[:, wb + wc:wb + wc + n]))
                # bias enters last so aux stays off the group-start path
                mms.append((ps[:, 0:GW], ones_t, bias_t[:, g * GW:(g + 1) * GW]))
                last = len(mms) - 1
                for idx, (o, lh, rh) in enumerate(mms):
                    nc.tensor.matmul(o, lh, rh,
                                     start=(idx == 0), stop=(idx == last))
                nc.vector.tensor_copy(
                    out_sb[:, g * GW:(g + 1) * GW], ps[:])
                nc.scalar.dma_start(out[g * BS:(g + 1) * BS, :],
                                    out_sb[:, g * GW:(g + 1) * GW])
    nc.compile()
    return nc


_NC_CACHE = []


def _get_nc():
    if not _NC_CACHE:
        _NC_CACHE.append(_build_nc())
    return _NC_CACHE[0]


def make_in_maps(x, W, bias):
    x = np.asarray(x, np.float32)
    wt, bsv = prep_weights(W, bias)
    wt16 = wt.astype(BF16)
    auxv = np.concatenate(
        [bsv.astype(BF16), np.ones((1, BS), BF16)], axis=1)
    in_maps = []
    for c in range(NCORES):
        xc = x[c * BS:(c + 1) * BS]                      # [b, r, j']
        xtv = np.ascontiguousarray(
            xc.transpose(2, 1, 0)).astype(BF16).reshape(R, R * BS)
        parts = []
        for ic in range(NCH):
            parts.append(xtv[:, ic * CHUNK * BS:(ic + 1) * CHUNK * BS])
            parts.append(wt16[:, ic * CHUNK * 50:(ic + 1) * CHUNK * 50])
        xwv = np.ascontiguousarray(np.concatenate(parts, axis=1))
        in_maps.append({"xw": xwv, "aux": auxv})
    return in_maps


def run(x, W, bias, trace=False, **kw):
    from concourse import bass_utils
    nc = _get_nc()
    res = bass_utils.run_bass_kernel_spmd(
        nc, make_in_maps(x, W, bias), list(range(NCORES)), trace=trace, **kw)
    outs = []
    for c in range(NCORES):
        o = np.asarray(res.results[c]["out"])            # [NG*BS, GW] bf16
        o = o.reshape(NG, BS, GW).transpose(1, 0, 2)     # [BS, NG, GW]
        outs.append(o.reshape(BS, 25, 25).astype(np.float32))
    return np.concatenate(outs, axis=0), res


def kernel(**inputs):
    out, _ = run(inputs["x"], inputs["W"], inputs["bias"])
    return out
